# revision 3
# baseline (speedup 1.0000x reference)
"""Chamfer loss kernel for Trainium2 (8 NeuronCores, SPMD).

Problem: preds [8, 8192, 3] f32, gts [8, 8192, 3] f32.
  P[b]   = pairwise sq-dists between gts[b] (rows m) and preds[b] (cols n)
  loss   = mean_n min_m P + mean_m min_n P   (means over all b,n / b,m)

Strategy: one batch element per core. Per core, the [8192, 8192] distance
matrix is produced tile-by-tile on the TensorEngine as a K=5 matmul:
  lhsT rows = [-2*gx, -2*gy, -2*gz, 1, 1]        (fp16, stationary, per m-tile)
  rhs  rows = [ px,    py,    pz,   yy_hi, yy_lo] (fp16, moving, per n-tile)
so PSUM gets  -2*g.p + |p|^2 (split hi/lo for exactness)  in fp32.
ScalarE adds |g|^2 as an exact fp32 per-partition bias while converting the
tile to fp16 in SBUF. VectorE then runs the two min-reduction paths in fp16
(2x DVE mode): a column accumulator (min over m-blocks, elementwise) and a
row-tile accumulator (min over n-blocks) + free-axis reduce. The final
partition-axis min of the column accumulator goes through PE transposes.
Host sums the 8 per-core scalars and divides by B*N.

Because both point sets pass through fp16 consistently, the computed values
are |g~ - p~|^2 of the fp16-rounded points (the big |g|^2,|p|^2 / cross
terms cancel exactly); error vs fp32 reference ~1e-4 absolute on the mins.
"""

import os
import sys

import numpy as np

for _p in ("/opt/trn_rl_repo",):
    if _p not in sys.path and os.path.isdir(_p):
        sys.path.insert(0, _p)

B = 8
NPTS = 8192  # both M (gts) and N (preds)
D = 3
PB = 128  # partition block (m-tile)
FD = 512  # free-dim block (n-tile)
MB = NPTS // PB  # 64 m-blocks
NB = NPTS // FD  # 16 n-blocks
NCH = NPTS // PB  # 64 transpose chunks of the column accumulator

_CACHE = {}


def _build(mb_blocks, nb_blocks):
    """Build and compile the per-core Bass module. Returns the Bacc object."""
    from contextlib import ExitStack

    import concourse.bass as bass
    import concourse.tile as tile
    from concourse import bacc, mybir

    f16 = mybir.dt.float16
    f32 = mybir.dt.float32
    M = mb_blocks * PB
    NP = nb_blocks * FD
    nch = NP // PB

    nc = bacc.Bacc(
        "TRN2",
        target_bir_lowering=False,
        debug=False,
        enable_asserts=False,
        num_devices=8,
    )

    glhs_d = nc.dram_tensor("glhs", [5, M], f16, kind="ExternalInput").ap()
    prhs_d = nc.dram_tensor("prhs", [5, NP], f16, kind="ExternalInput").ap()
    xx_d = nc.dram_tensor("xx", [PB, mb_blocks], f32, kind="ExternalInput").ap()
    ident_d = nc.dram_tensor("ident", [PB, PB], f16, kind="ExternalInput").ap()
    ones_d = nc.dram_tensor("ones", [PB, 1], f32, kind="ExternalInput").ap()
    out_d = nc.dram_tensor("out", [1, 1], f32, kind="ExternalOutput").ap()

    def body(ctx: ExitStack, tc: tile.TileContext):
        nc = tc.nc
        const_pool = ctx.enter_context(tc.tile_pool(name="const", bufs=1))
        acc_pool = ctx.enter_context(tc.tile_pool(name="acc", bufs=1))
        work_pool = ctx.enter_context(tc.tile_pool(name="work", bufs=6))
        psum_pool = ctx.enter_context(tc.tile_pool(name="psum", bufs=4, space="PSUM"))

        glhs_sb = const_pool.tile([5, M], f16)
        nc.sync.dma_start(glhs_sb[:], glhs_d[:])
        prhs_sb = const_pool.tile([5, NP], f16)
        nc.sync.dma_start(prhs_sb[:], prhs_d[:])
        xx_sb = const_pool.tile([PB, mb_blocks], f32)
        nc.sync.dma_start(xx_sb[:], xx_d[:])
        ident_sb = const_pool.tile([PB, PB], f16)
        nc.sync.dma_start(ident_sb[:], ident_d[:])
        ones_sb = const_pool.tile([PB, 1], f32)
        nc.sync.dma_start(ones_sb[:], ones_d[:])

        colacc = acc_pool.tile([PB, NP], f16)
        rowmins = acc_pool.tile([PB, mb_blocks], f32)
        colmins = acc_pool.tile([PB, nch], f32)

        ident_act = mybir.ActivationFunctionType.Identity

        for mb in range(mb_blocks):
            rowtile = work_pool.tile([PB, FD], f16, tag="rowtile", bufs=2)
            for nb in range(nb_blocks):
                ps = psum_pool.tile([PB, FD], f32, tag="ps")
                nc.tensor.matmul(
                    ps[:],
                    glhs_sb[:, mb * PB : (mb + 1) * PB],
                    prhs_sb[:, nb * FD : (nb + 1) * FD],
                    start=True,
                    stop=True,
                )
                # fp16 conversion + exact fp32 row bias |g|^2
                if nb == 0:
                    dst = rowtile
                else:
                    dst = work_pool.tile([PB, FD], f16, tag="stile", bufs=4)
                nc.scalar.activation(
                    dst[:], ps[:], ident_act, bias=xx_sb[:, mb : mb + 1], scale=1.0
                )
                # column path: elementwise min over m-blocks
                csl = colacc[:, nb * FD : (nb + 1) * FD]
                if mb == 0:
                    nc.vector.tensor_copy(csl, dst[:])
                else:
                    nc.vector.tensor_tensor(csl, csl, dst[:], mybir.AluOpType.min)
                # row path: elementwise min over n-blocks
                if nb > 0:
                    nc.vector.tensor_tensor(
                        rowtile[:], rowtile[:], dst[:], mybir.AluOpType.min
                    )
            nc.vector.tensor_reduce(
                rowmins[:, mb : mb + 1],
                rowtile[:],
                axis=mybir.AxisListType.X,
                op=mybir.AluOpType.min,
            )

        # partition-axis min of colacc via PE transposes
        for ch in range(nch):
            tps = psum_pool.tile([PB, PB], f16, tag="tps", bufs=2)
            nc.tensor.transpose(
                tps[:], colacc[:, ch * PB : (ch + 1) * PB], ident_sb[:]
            )
            nc.vector.tensor_reduce(
                colmins[:, ch : ch + 1],
                tps[:],
                axis=mybir.AxisListType.X,
                op=mybir.AluOpType.min,
            )

        rsum = acc_pool.tile([PB, 1], f32)
        nc.vector.tensor_reduce(
            rsum[:], rowmins[:], axis=mybir.AxisListType.X, op=mybir.AluOpType.add
        )
        csum = acc_pool.tile([PB, 1], f32)
        nc.vector.tensor_reduce(
            csum[:], colmins[:], axis=mybir.AxisListType.X, op=mybir.AluOpType.add
        )
        tot = acc_pool.tile([PB, 1], f32)
        nc.vector.tensor_add(tot[:], rsum[:], csum[:])
        outp = psum_pool.tile([1, 1], f32, tag="outp", bufs=1)
        nc.tensor.matmul(outp[:], tot[:], ones_sb[:], start=True, stop=True)
        out_sb = acc_pool.tile([1, 1], f32)
        nc.vector.tensor_copy(out_sb[:], outp[:])
        nc.sync.dma_start(out_d[:], out_sb[:])

    with tile.TileContext(nc) as tc:
        with ExitStack() as ctx:
            body(ctx, tc)

    nc.compile()
    return nc


def _get_nc(mb_blocks=MB, nb_blocks=NB):
    key = (mb_blocks, nb_blocks)
    if key not in _CACHE:
        _CACHE[key] = _build(mb_blocks, nb_blocks)
    return _CACHE[key]


def _prep_core(g, p, mb_blocks, nb_blocks):
    """Host-side prep of one core's input arrays from gts[b], preds[b]."""
    m = mb_blocks * PB
    n = nb_blocks * FD
    g16 = g[:m].astype(np.float16)
    p16 = p[:n].astype(np.float16)
    g32 = g16.astype(np.float32)
    p32 = p16.astype(np.float32)

    glhs = np.empty((5, m), np.float16)
    glhs[0:3] = (-2.0 * g32.T).astype(np.float16)  # exact: *2 bumps exponent
    glhs[3:5] = np.float16(1.0)

    yy = (p32 * p32).sum(-1, dtype=np.float32)
    yy_hi = yy.astype(np.float16)
    yy_lo = (yy - yy_hi.astype(np.float32)).astype(np.float16)
    prhs = np.empty((5, n), np.float16)
    prhs[0:3] = p16.T
    prhs[3] = yy_hi
    prhs[4] = yy_lo

    xx = (g32 * g32).sum(-1, dtype=np.float32)
    xx_in = np.ascontiguousarray(xx.reshape(mb_blocks, PB).T)  # [128, MB]

    return {
        "glhs": glhs,
        "prhs": prhs,
        "xx": xx_in,
        "ident": np.eye(PB, dtype=np.float16),
        "ones": np.ones((PB, 1), np.float32),
    }


def kernel(preds, gts):
    preds = np.asarray(preds)
    gts = np.asarray(gts)
    assert preds.shape == (B, NPTS, D) and gts.shape == (B, NPTS, D)

    from concourse.bass_utils import run_bass_kernel_spmd

    nc = _get_nc()
    in_maps = [_prep_core(gts[b], preds[b], MB, NB) for b in range(B)]
    res = run_bass_kernel_spmd(nc, in_maps, list(range(B)))
    total = sum(float(r["out"][0, 0]) for r in res.results)
    # loss = sum(colmins)/(B*N) + sum(rowmins)/(B*M), N == M == NPTS
    return np.float32(total / (B * NPTS))


# revision 21
# speedup vs baseline: 1.2531x; 1.2531x over previous
"""Chamfer loss kernel for Trainium2 (8 NeuronCores, SPMD).

Problem: preds [8, 8192, 3] f32, gts [8, 8192, 3] f32.
  P[b]   = pairwise sq-dists between gts[b] (rows m) and preds[b] (cols n)
  loss   = mean_n min_m P + mean_m min_n P   (means over all b,n / b,m)

Strategy: one batch element per core. Per core, the [8192, 8192] distance
matrix is produced tile-by-tile on the TensorEngine as a K=5 matmul:
  lhsT rows = [-2*gx, -2*gy, -2*gz, 1, 1]        (fp16, stationary, per m-tile)
  rhs  rows = [ px,    py,    pz,   yy_hi, yy_lo] (fp16, moving, per n-tile)
so PSUM gets  -2*g.p + |p|^2 (split hi/lo for exactness)  in fp32.
ScalarE adds |g|^2 as an exact fp32 per-partition bias while converting the
tile to fp16 in SBUF. VectorE then runs the two min-reduction paths in fp16
(2x DVE mode): a column accumulator (min over m-blocks, elementwise) and a
row-tile accumulator (min over n-blocks) + free-axis reduce. The final
partition-axis min of the column accumulator goes through PE transposes.
Host sums the 8 per-core scalars and divides by B*N.

Because both point sets pass through fp16 consistently, the computed values
are |g~ - p~|^2 of the fp16-rounded points (the big |g|^2,|p|^2 / cross
terms cancel exactly); error vs fp32 reference ~1e-4 absolute on the mins.
"""

import os
import sys

import numpy as np

for _p in ("/opt/trn_rl_repo",):
    if _p not in sys.path and os.path.isdir(_p):
        sys.path.insert(0, _p)

B = 8
NPTS = 8192  # both M (gts) and N (preds)
D = 3
PB = 128  # partition block (m-tile)
FD = 512  # free-dim block (n-tile)
MB = NPTS // PB  # 64 m-blocks
NB = NPTS // FD  # 16 n-blocks
NCH = NPTS // PB  # 64 transpose chunks of the column accumulator

_CACHE = {}


def _build(
    mb_blocks, nb_blocks, loop=1, col_offload="none", col_k=4, psg=4, use_ttr=False
):
    # use_ttr=True (fused tensor_tensor_reduce) crashes the device through
    # this NEFF path — the custom DVE uop table isn't shipped. Keep False.
    """Build and compile the per-core Bass module. Returns the Bacc object.

    loop > 1 wraps the whole body in a hardware For_i — used only for
    timing (device time ~ loop * kernel time, amortizes dispatch noise).
    col_offload: route every col_k-th m-block's column-min to another
    engine: "gtt" = gpsimd tensor_tensor, "gdma" = gpsimd compute-DMA.
    """
    from contextlib import ExitStack

    import concourse.bass as bass
    import concourse.tile as tile
    from concourse import bacc, mybir

    f16 = mybir.dt.float16
    f32 = mybir.dt.float32
    M = mb_blocks * PB
    NP = nb_blocks * FD
    nch = NP // PB

    nc = bacc.Bacc(
        "TRN2",
        target_bir_lowering=False,
        debug=False,
        enable_asserts=False,
        num_devices=8,
    )

    glhs_d = nc.dram_tensor("glhs", [5, M], f16, kind="ExternalInput").ap()
    prhs_d = nc.dram_tensor("prhs", [5, NP], f16, kind="ExternalInput").ap()
    xx_d = nc.dram_tensor("xx", [PB, mb_blocks], f32, kind="ExternalInput").ap()
    ident_d = nc.dram_tensor("ident", [PB, PB], f16, kind="ExternalInput").ap()
    ones_d = nc.dram_tensor("ones", [PB, 1], f32, kind="ExternalInput").ap()
    out_d = nc.dram_tensor("out", [1, 1], f32, kind="ExternalOutput").ap()

    def body(ctx: ExitStack, tc: tile.TileContext):
        nc = tc.nc
        const_pool = ctx.enter_context(tc.tile_pool(name="const", bufs=1))
        acc_pool = ctx.enter_context(tc.tile_pool(name="acc", bufs=1))
        work_pool = ctx.enter_context(tc.tile_pool(name="work", bufs=2))
        psum_pool = ctx.enter_context(tc.tile_pool(name="psum", bufs=2, space="PSUM"))

        glhs_sb = const_pool.tile([5, M], f16)
        nc.sync.dma_start(glhs_sb[:], glhs_d[:])
        prhs_sb = const_pool.tile([5, NP], f16)
        nc.sync.dma_start(prhs_sb[:], prhs_d[:])
        xx_sb = const_pool.tile([PB, mb_blocks], f32)
        nc.sync.dma_start(xx_sb[:], xx_d[:])
        ident_sb = const_pool.tile([PB, PB], f16)
        nc.sync.dma_start(ident_sb[:], ident_d[:])
        ones_sb = const_pool.tile([PB, 1], f32)
        nc.sync.dma_start(ones_sb[:], ones_d[:])

        colacc = acc_pool.tile([PB, NP], f16)
        colaccB = (
            acc_pool.tile([PB, NP], f16, name="colaccB")
            if col_offload == "split"
            else None
        )
        rowmins = acc_pool.tile([PB, mb_blocks], f32)
        colmins = acc_pool.tile([PB, nch], f32)

        ident_act = mybir.ActivationFunctionType.Identity
        amin = mybir.AluOpType.min

        PSG = min(psg, nb_blocks)  # psum banks (512-col matmuls) per ACT op
        assert nb_blocks % PSG == 0

        for mb in range(mb_blocks):
            # one fp16 row-buffer holding this m-block's full distance row
            rowbuf = work_pool.tile([PB, NP], f16, tag="rowbuf", bufs=2)
            for nbg in range(nb_blocks // PSG):
                ps = psum_pool.tile([PB, PSG * FD], f32, tag="ps")
                for j in range(PSG):
                    nb = nbg * PSG + j
                    nc.tensor.matmul(
                        ps[:, j * FD : (j + 1) * FD],
                        glhs_sb[:, mb * PB : (mb + 1) * PB],
                        prhs_sb[:, nb * FD : (nb + 1) * FD],
                        start=True,
                        stop=True,
                    )
                # fp16 conversion + exact fp32 row bias |g|^2
                nc.scalar.activation(
                    rowbuf[:, nbg * PSG * FD : (nbg + 1) * PSG * FD],
                    ps[:],
                    ident_act,
                    bias=xx_sb[:, mb : mb + 1],
                    scale=1.0,
                )
            # column path: one elementwise min over the whole row-buffer
            if col_offload == "split" and mb % 2 == 1:
                # independent second chain on compute-DMA (SDMA CCE min)
                if mb == 1:
                    nc.gpsimd.dma_start(colaccB[:], rowbuf[:])
                else:
                    nc.gpsimd.dma_start(colaccB[:], rowbuf[:], accum_op=amin)
            elif mb == 0:
                nc.vector.tensor_copy(colacc[:], rowbuf[:])
            elif col_offload in ("gtt", "gdma") and mb % col_k == col_k - 1:
                if col_offload == "gtt":
                    nc.gpsimd.tensor_tensor(colacc[:], colacc[:], rowbuf[:], amin)
                else:
                    nc.gpsimd.dma_start(colacc[:], rowbuf[:], accum_op=amin)
            else:
                nc.vector.tensor_tensor(colacc[:], colacc[:], rowbuf[:], amin)
            # row path: in-place fp16 min-tree, final level fused with reduce
            scr = work_pool.tile([PB, NP // 2], f16, tag="scr", bufs=2)
            nc.vector.tensor_tensor(
                scr[:], rowbuf[:, : NP // 2], rowbuf[:, NP // 2 :], amin
            )
            w = NP // 4
            while w > FD:
                nc.vector.tensor_tensor(
                    scr[:, :w], scr[:, :w], scr[:, w : 2 * w], amin
                )
                w //= 2
            if use_ttr:
                nc.vector.tensor_tensor_reduce(
                    out=scr[:, :w],
                    in0=scr[:, :w],
                    in1=scr[:, w : 2 * w],
                    scale=1.0,
                    scalar=60000.0,
                    op0=amin,
                    op1=amin,
                    accum_out=rowmins[:, mb : mb + 1],
                )
            else:
                nc.vector.tensor_tensor(
                    scr[:, :w], scr[:, :w], scr[:, w : 2 * w], amin
                )
                nc.vector.tensor_reduce(
                    rowmins[:, mb : mb + 1],
                    scr[:, :w],
                    axis=mybir.AxisListType.X,
                    op=amin,
                )

        # merge the second column chain (if split)
        if colaccB is not None:
            nc.vector.tensor_tensor(colacc[:], colacc[:], colaccB[:], amin)

        # partition-axis min of colacc via PE transposes
        for ch in range(nch):
            tps = psum_pool.tile([PB, PB], f16, tag="ps", bufs=2)
            nc.tensor.transpose(
                tps[:], colacc[:, ch * PB : (ch + 1) * PB], ident_sb[:]
            )
            nc.vector.tensor_reduce(
                colmins[:, ch : ch + 1],
                tps[:],
                axis=mybir.AxisListType.X,
                op=mybir.AluOpType.min,
            )

        rsum = acc_pool.tile([PB, 1], f32)
        nc.vector.tensor_reduce(
            rsum[:], rowmins[:], axis=mybir.AxisListType.X, op=mybir.AluOpType.add
        )
        csum = acc_pool.tile([PB, 1], f32)
        nc.vector.tensor_reduce(
            csum[:], colmins[:], axis=mybir.AxisListType.X, op=mybir.AluOpType.add
        )
        tot = acc_pool.tile([PB, 1], f32)
        nc.vector.tensor_add(tot[:], rsum[:], csum[:])
        outp = psum_pool.tile([1, 1], f32, tag="ps", bufs=2)
        nc.tensor.matmul(outp[:], tot[:], ones_sb[:], start=True, stop=True)
        out_sb = acc_pool.tile([1, 1], f32)
        nc.vector.tensor_copy(out_sb[:], outp[:])
        nc.sync.dma_start(out_d[:], out_sb[:])

    with tile.TileContext(nc) as tc:
        with ExitStack() as ctx:
            if loop > 1:
                with tc.For_i(0, loop, 1):
                    body(ctx, tc)
            else:
                body(ctx, tc)

    nc.compile()
    return nc


def _get_nc(mb_blocks=MB, nb_blocks=NB):
    key = (mb_blocks, nb_blocks)
    if key not in _CACHE:
        _CACHE[key] = _build(mb_blocks, nb_blocks)
    return _CACHE[key]


def _prep_core(g, p, mb_blocks, nb_blocks):
    """Host-side prep of one core's input arrays from gts[b], preds[b]."""
    m = mb_blocks * PB
    n = nb_blocks * FD
    g16 = g[:m].astype(np.float16)
    p16 = p[:n].astype(np.float16)
    g32 = g16.astype(np.float32)
    p32 = p16.astype(np.float32)

    glhs = np.empty((5, m), np.float16)
    glhs[0:3] = (-2.0 * g32.T).astype(np.float16)  # exact: *2 bumps exponent
    glhs[3:5] = np.float16(1.0)

    yy = (p32 * p32).sum(-1, dtype=np.float32)
    yy_hi = yy.astype(np.float16)
    yy_lo = (yy - yy_hi.astype(np.float32)).astype(np.float16)
    prhs = np.empty((5, n), np.float16)
    prhs[0:3] = p16.T
    prhs[3] = yy_hi
    prhs[4] = yy_lo

    xx = (g32 * g32).sum(-1, dtype=np.float32)
    xx_in = np.ascontiguousarray(xx.reshape(mb_blocks, PB).T)  # [128, MB]

    return {
        "glhs": glhs,
        "prhs": prhs,
        "xx": xx_in,
        "ident": np.eye(PB, dtype=np.float16),
        "ones": np.ones((PB, 1), np.float32),
    }


def kernel(preds, gts):
    preds = np.asarray(preds)
    gts = np.asarray(gts)
    assert preds.shape == (B, NPTS, D) and gts.shape == (B, NPTS, D)

    from concourse.bass_utils import run_bass_kernel_spmd

    nc = _get_nc()
    in_maps = [_prep_core(gts[b], preds[b], MB, NB) for b in range(B)]
    res = run_bass_kernel_spmd(nc, in_maps, list(range(B)))
    total = sum(float(r["out"][0, 0]) for r in res.results)
    # loss = sum(colmins)/(B*N) + sum(rowmins)/(B*M), N == M == NPTS
    return np.float32(total / (B * NPTS))


# revision 33
# speedup vs baseline: 271.1293x; 216.3584x over previous
"""Chamfer loss kernel for Trainium2 (8 NeuronCores, SPMD).

Problem: preds [8, 8192, 3] f32, gts [8, 8192, 3] f32.
  P[b]   = pairwise sq-dists between gts[b] (rows m) and preds[b] (cols n)
  loss   = mean_n min_m P + mean_m min_n P   (means over all b,n / b,m)

Strategy: one batch element per core. Per core, the [8192, 8192] distance
matrix is produced tile-by-tile on the TensorEngine as a K=5 matmul:
  lhsT rows = [-2*gx, -2*gy, -2*gz, 1, 1]        (fp16, stationary, per m-tile)
  rhs  rows = [ px,    py,    pz,   yy_hi, yy_lo] (fp16, moving, per n-tile)
so PSUM gets  -2*g.p + |p|^2 (split hi/lo for exactness)  in fp32.
ScalarE adds |g|^2 as an exact fp32 per-partition bias while converting the
tile to fp16 in SBUF. VectorE then runs the two min-reduction paths in fp16
(2x DVE mode): a column accumulator (min over m-blocks, elementwise) and a
row-tile accumulator (min over n-blocks) + free-axis reduce. The final
partition-axis min of the column accumulator goes through PE transposes.
Host sums the 8 per-core scalars and divides by B*N.

Because both point sets pass through fp16 consistently, the computed values
are |g~ - p~|^2 of the fp16-rounded points (the big |g|^2,|p|^2 / cross
terms cancel exactly); error vs fp32 reference ~1e-4 absolute on the mins.
"""

import os
import sys

import numpy as np

for _p in ("/opt/trn_rl_repo",):
    if _p not in sys.path and os.path.isdir(_p):
        sys.path.insert(0, _p)

B = 8
NPTS = 8192  # both M (gts) and N (preds)
D = 3
PB = 128  # partition block (m-tile)
FD = 512  # free-dim block (n-tile)
MB = NPTS // PB  # 64 m-blocks
NB = NPTS // FD  # 16 n-blocks
NCH = NPTS // PB  # 64 transpose chunks of the column accumulator

_CACHE = {}


def _build(
    mb_blocks,
    nb_blocks,
    loop=1,
    col_offload="none",
    col_k=4,
    psg=4,
    use_ttr=False,
    skip_col=False,
    skip_row=False,
    skip_act=False,
    bias_imm=False,
    classic=False,
    rowgrp=True,
):
    # use_ttr=True (fused tensor_tensor_reduce) crashes the device through
    # this NEFF path — the custom DVE uop table isn't shipped. Keep False.
    """Build and compile the per-core Bass module. Returns the Bacc object.

    loop > 1 wraps the whole body in a hardware For_i — used only for
    timing (device time ~ loop * kernel time, amortizes dispatch noise).
    col_offload: route every col_k-th m-block's column-min to another
    engine: "gtt" = gpsimd tensor_tensor, "gdma" = gpsimd compute-DMA.
    """
    from contextlib import ExitStack

    import concourse.bass as bass
    import concourse.tile as tile
    from concourse import bacc, mybir

    f16 = mybir.dt.float16
    f32 = mybir.dt.float32
    M = mb_blocks * PB
    NP = nb_blocks * FD
    nch = NP // PB

    nc = bacc.Bacc(
        "TRN2",
        target_bir_lowering=False,
        debug=False,
        enable_asserts=False,
        num_devices=8,
    )

    glhs_d = nc.dram_tensor("glhs", [5, M], f16, kind="ExternalInput").ap()
    prhs_d = nc.dram_tensor("prhs", [5, NP], f16, kind="ExternalInput").ap()
    xx_d = nc.dram_tensor("xx", [PB, mb_blocks], f32, kind="ExternalInput").ap()
    ident_d = nc.dram_tensor("ident", [PB, PB], f16, kind="ExternalInput").ap()
    ones_d = nc.dram_tensor("ones", [PB, 1], f32, kind="ExternalInput").ap()
    out_d = nc.dram_tensor("out", [1, 1], f32, kind="ExternalOutput").ap()

    def body(ctx: ExitStack, tc: tile.TileContext):
        nc = tc.nc
        const_pool = ctx.enter_context(tc.tile_pool(name="const", bufs=1))
        acc_pool = ctx.enter_context(tc.tile_pool(name="acc", bufs=1))
        work_pool = ctx.enter_context(tc.tile_pool(name="work", bufs=2))
        psum_pool = ctx.enter_context(tc.tile_pool(name="psum", bufs=2, space="PSUM"))

        # K=5 operands replicated at partitions {0,32,64,96} so four
        # matmuls can run concurrently in distinct PE row groups
        glhs_sb = const_pool.tile([128, M], f16)
        prhs_sb = const_pool.tile([128, NP], f16)
        for r in range(4):
            nc.sync.dma_start(glhs_sb[32 * r : 32 * r + 5, :], glhs_d[:])
            nc.sync.dma_start(prhs_sb[32 * r : 32 * r + 5, :], prhs_d[:])
        xx_sb = const_pool.tile([PB, mb_blocks], f32)
        nc.sync.dma_start(xx_sb[:], xx_d[:])
        ident_sb = const_pool.tile([PB, PB], f16)
        nc.sync.dma_start(ident_sb[:], ident_d[:])
        ones_sb = const_pool.tile([PB, 1], f32)
        nc.sync.dma_start(ones_sb[:], ones_d[:])

        colacc = acc_pool.tile([PB, NP], f16)
        colaccB = (
            acc_pool.tile([PB, NP], f16, name="colaccB")
            if col_offload == "split"
            else None
        )
        rowmins = acc_pool.tile([PB, mb_blocks], f32)
        colmins = acc_pool.tile([PB, nch], f32)

        ident_act = mybir.ActivationFunctionType.Identity
        amin = mybir.AluOpType.min

        def mk_mm(ps_slice, mb, nb, j):
            r = j % 4 if rowgrp else 0
            nc.tensor.matmul(
                ps_slice,
                glhs_sb[32 * r : 32 * r + 5, mb * PB : (mb + 1) * PB],
                prhs_sb[32 * r : 32 * r + 5, nb * FD : (nb + 1) * FD],
                start=True,
                stop=True,
                tile_position=(32 * r, 0),
            )

        if classic:
            # v1 pipeline: per-512-tile ACT conversion + per-tile DVE mins
            for mb in range(mb_blocks):
                rowtile = work_pool.tile([PB, FD], f16, tag="rowtile", bufs=2)
                for nb in range(nb_blocks):
                    ps = psum_pool.tile([PB, FD], f32, tag="cps", bufs=4)
                    mk_mm(ps[:], mb, nb, nb)
                    if nb == 0:
                        dst = rowtile
                    else:
                        dst = work_pool.tile([PB, FD], f16, tag="stile", bufs=4)
                    nc.scalar.activation(
                        dst[:], ps[:], mybir.ActivationFunctionType.Identity,
                        bias=xx_sb[:, mb : mb + 1], scale=1.0,
                    )
                    csl = colacc[:, nb * FD : (nb + 1) * FD]
                    if mb == 0:
                        nc.vector.tensor_copy(csl, dst[:])
                    else:
                        nc.vector.tensor_tensor(csl, csl, dst[:], amin)
                    if nb > 0:
                        nc.vector.tensor_tensor(
                            rowtile[:], rowtile[:], dst[:], amin
                        )
                nc.vector.tensor_reduce(
                    rowmins[:, mb : mb + 1], rowtile[:],
                    axis=mybir.AxisListType.X, op=amin,
                )

        PSG = min(psg, nb_blocks)  # psum banks (512-col matmuls) per ACT op
        assert nb_blocks % PSG == 0

        for mb in range(mb_blocks if not classic else 0):
            # one fp16 row-buffer holding this m-block's full distance row
            rowbuf = work_pool.tile([PB, NP], f16, tag="rowbuf", bufs=2)
            for nbg in range(nb_blocks // PSG):
                ps = psum_pool.tile([PB, PSG * FD], f32, tag="ps")
                for j in range(PSG):
                    nb = nbg * PSG + j
                    r = j % 4
                    nc.tensor.matmul(
                        ps[:, j * FD : (j + 1) * FD],
                        glhs_sb[32 * r : 32 * r + 5, mb * PB : (mb + 1) * PB],
                        prhs_sb[32 * r : 32 * r + 5, nb * FD : (nb + 1) * FD],
                        start=True,
                        stop=True,
                        tile_position=(32 * r, 0),
                    )
                # fp16 conversion + exact fp32 row bias |g|^2
                if not skip_act:
                    if bias_imm:
                        nc.scalar.activation(
                            rowbuf[:, nbg * PSG * FD : (nbg + 1) * PSG * FD],
                            ps[:],
                            mybir.ActivationFunctionType.Copy,
                            bias=0.0,
                            scale=1.0,
                        )
                    else:
                        nc.scalar.activation(
                            rowbuf[:, nbg * PSG * FD : (nbg + 1) * PSG * FD],
                            ps[:],
                            ident_act,
                            bias=xx_sb[:, mb : mb + 1],
                            scale=1.0,
                        )
            # column path: one elementwise min over the whole row-buffer
            if skip_col:
                pass
            elif col_offload == "split" and mb % 2 == 1:
                # independent second chain on compute-DMA (SDMA CCE min)
                if mb == 1:
                    nc.gpsimd.dma_start(colaccB[:], rowbuf[:])
                else:
                    nc.gpsimd.dma_start(colaccB[:], rowbuf[:], accum_op=amin)
            elif mb == 0:
                nc.vector.tensor_copy(colacc[:], rowbuf[:])
            elif col_offload in ("gtt", "gdma") and mb % col_k == col_k - 1:
                if col_offload == "gtt":
                    nc.gpsimd.tensor_tensor(colacc[:], colacc[:], rowbuf[:], amin)
                else:
                    nc.gpsimd.dma_start(colacc[:], rowbuf[:], accum_op=amin)
            else:
                nc.vector.tensor_tensor(colacc[:], colacc[:], rowbuf[:], amin)
            # row path: in-place fp16 min-tree, final level fused with reduce
            if skip_row:
                continue
            scr = work_pool.tile([PB, NP // 2], f16, tag="scr", bufs=2)
            nc.vector.tensor_tensor(
                scr[:], rowbuf[:, : NP // 2], rowbuf[:, NP // 2 :], amin
            )
            w = NP // 4
            while w > FD:
                nc.vector.tensor_tensor(
                    scr[:, :w], scr[:, :w], scr[:, w : 2 * w], amin
                )
                w //= 2
            if use_ttr:
                nc.vector.tensor_tensor_reduce(
                    out=scr[:, :w],
                    in0=scr[:, :w],
                    in1=scr[:, w : 2 * w],
                    scale=1.0,
                    scalar=60000.0,
                    op0=amin,
                    op1=amin,
                    accum_out=rowmins[:, mb : mb + 1],
                )
            else:
                nc.vector.tensor_tensor(
                    scr[:, :w], scr[:, :w], scr[:, w : 2 * w], amin
                )
                nc.vector.tensor_reduce(
                    rowmins[:, mb : mb + 1],
                    scr[:, :w],
                    axis=mybir.AxisListType.X,
                    op=amin,
                )

        # merge the second column chain (if split)
        if colaccB is not None and not skip_col:
            nc.vector.tensor_tensor(colacc[:], colacc[:], colaccB[:], amin)

        # partition-axis min of colacc via PE transposes
        if not skip_col and not skip_act:
            for ch in range(nch):
                tps = psum_pool.tile([PB, PB], f16, tag="ps", bufs=2)
                nc.tensor.transpose(
                    tps[:], colacc[:, ch * PB : (ch + 1) * PB], ident_sb[:]
                )
                nc.vector.tensor_reduce(
                    colmins[:, ch : ch + 1],
                    tps[:],
                    axis=mybir.AxisListType.X,
                    op=mybir.AluOpType.min,
                )

        rsum = acc_pool.tile([PB, 1], f32)
        if not skip_row and not skip_act:
            nc.vector.tensor_reduce(
                rsum[:],
                rowmins[:],
                axis=mybir.AxisListType.X,
                op=mybir.AluOpType.add,
            )
        else:
            nc.vector.tensor_copy(rsum[:], ones_sb[:])
        csum = acc_pool.tile([PB, 1], f32)
        if not skip_col and not skip_act:
            nc.vector.tensor_reduce(
                csum[:],
                colmins[:],
                axis=mybir.AxisListType.X,
                op=mybir.AluOpType.add,
            )
        else:
            nc.vector.tensor_copy(csum[:], ones_sb[:])
        tot = acc_pool.tile([PB, 1], f32)
        nc.vector.tensor_add(tot[:], rsum[:], csum[:])
        outp = psum_pool.tile([1, 1], f32, tag="ps", bufs=2)
        nc.tensor.matmul(outp[:], tot[:], ones_sb[:], start=True, stop=True)
        out_sb = acc_pool.tile([1, 1], f32)
        nc.vector.tensor_copy(out_sb[:], outp[:])
        nc.sync.dma_start(out_d[:], out_sb[:])

    with tile.TileContext(nc) as tc:
        with ExitStack() as ctx:
            if loop > 1:
                with tc.For_i(0, loop, 1):
                    body(ctx, tc)
            else:
                body(ctx, tc)

    nc.compile()
    return nc


def _get_nc(mb_blocks=MB, nb_blocks=NB):
    key = (mb_blocks, nb_blocks)
    if key not in _CACHE:
        _CACHE[key] = _build(mb_blocks, nb_blocks)
    return _CACHE[key]


def _prep_core(g, p, mb_blocks, nb_blocks):
    """Host-side prep of one core's input arrays from gts[b], preds[b]."""
    m = mb_blocks * PB
    n = nb_blocks * FD
    g16 = g[:m].astype(np.float16)
    p16 = p[:n].astype(np.float16)
    g32 = g16.astype(np.float32)
    p32 = p16.astype(np.float32)

    glhs = np.empty((5, m), np.float16)
    glhs[0:3] = (-2.0 * g32.T).astype(np.float16)  # exact: *2 bumps exponent
    glhs[3:5] = np.float16(1.0)

    yy = (p32 * p32).sum(-1, dtype=np.float32)
    yy_hi = yy.astype(np.float16)
    yy_lo = (yy - yy_hi.astype(np.float32)).astype(np.float16)
    prhs = np.empty((5, n), np.float16)
    prhs[0:3] = p16.T
    prhs[3] = yy_hi
    prhs[4] = yy_lo

    xx = (g32 * g32).sum(-1, dtype=np.float32)
    xx_in = np.ascontiguousarray(xx.reshape(mb_blocks, PB).T)  # [128, MB]

    return {
        "glhs": glhs,
        "prhs": prhs,
        "xx": xx_in,
        "ident": np.eye(PB, dtype=np.float16),
        "ones": np.ones((PB, 1), np.float32),
    }


def kernel(preds, gts):
    preds = np.asarray(preds)
    gts = np.asarray(gts)
    assert preds.shape == (B, NPTS, D) and gts.shape == (B, NPTS, D)

    from concourse.bass_utils import run_bass_kernel_spmd

    nc = _get_nc()
    in_maps = [_prep_core(gts[b], preds[b], MB, NB) for b in range(B)]
    res = run_bass_kernel_spmd(nc, in_maps, list(range(B)))
    total = sum(float(r["out"][0, 0]) for r in res.results)
    # loss = sum(colmins)/(B*N) + sum(rowmins)/(B*M), N == M == NPTS
    return np.float32(total / (B * NPTS))


# revision 49
# speedup vs baseline: 458.0536x; 1.6894x over previous
"""Chamfer loss kernel for Trainium2 (8 NeuronCores, SPMD).

Problem: preds [8, 8192, 3] f32, gts [8, 8192, 3] f32.
  P[b]   = pairwise sq-dists between gts[b] (rows m) and preds[b] (cols n)
  loss   = mean_n min_m P + mean_m min_n P   (means over all b,n / b,m)

Strategy: one batch element per core. Per core, the [8192, 8192] distance
matrix is produced tile-by-tile on the TensorEngine as a K=5 matmul:
  lhsT rows = [-2*gx, -2*gy, -2*gz, 1, 1]        (fp16, stationary, per m-tile)
  rhs  rows = [ px,    py,    pz,   yy_hi, yy_lo] (fp16, moving, per n-tile)
so PSUM gets  -2*g.p + |p|^2 (split hi/lo for exactness)  in fp32.
ScalarE adds |g|^2 as an exact fp32 per-partition bias while converting the
tile to fp16 in SBUF. VectorE then runs the two min-reduction paths in fp16
(2x DVE mode): a column accumulator (min over m-blocks, elementwise) and a
row-tile accumulator (min over n-blocks) + free-axis reduce. The final
partition-axis min of the column accumulator goes through PE transposes.
Host sums the 8 per-core scalars and divides by B*N.

Because both point sets pass through fp16 consistently, the computed values
are |g~ - p~|^2 of the fp16-rounded points (the big |g|^2,|p|^2 / cross
terms cancel exactly); error vs fp32 reference ~1e-4 absolute on the mins.
"""

import os
import sys

import numpy as np

for _p in ("/opt/trn_rl_repo",):
    if _p not in sys.path and os.path.isdir(_p):
        sys.path.insert(0, _p)

B = 8
NPTS = 8192  # both M (gts) and N (preds)
D = 3
PB = 128  # partition block (m-tile)
FD = 512  # free-dim block (n-tile)
MB = NPTS // PB  # 64 m-blocks
NB = NPTS // FD  # 16 n-blocks
NCH = NPTS // PB  # 64 transpose chunks of the column accumulator

_CACHE = {}


def _build(
    mb_blocks,
    nb_blocks,
    loop=1,
    col_offload="none",
    col_k=4,
    psg=4,
    use_ttr=False,
    skip_col=False,
    skip_row=False,
    skip_act=False,
    bias_imm=False,
    classic=False,
    rowgrp=True,
    ps_bufs=2,
    work_bufs=2,
    interleave=False,
    preload=False,
    mm_n=FD,
    finegrain=False,
):
    # use_ttr=True (fused tensor_tensor_reduce) crashes the device through
    # this NEFF path — the custom DVE uop table isn't shipped. Keep False.
    """Build and compile the per-core Bass module. Returns the Bacc object.

    loop > 1 wraps the whole body in a hardware For_i — used only for
    timing (device time ~ loop * kernel time, amortizes dispatch noise).
    col_offload: route every col_k-th m-block's column-min to another
    engine: "gtt" = gpsimd tensor_tensor, "gdma" = gpsimd compute-DMA.
    """
    from contextlib import ExitStack

    import concourse.bass as bass
    import concourse.tile as tile
    from concourse import bacc, mybir

    f16 = mybir.dt.float16
    f32 = mybir.dt.float32
    M = mb_blocks * PB
    NP = nb_blocks * FD
    nch = NP // PB

    nc = bacc.Bacc(
        "TRN2",
        target_bir_lowering=False,
        debug=False,
        enable_asserts=False,
        num_devices=8,
    )

    glhs_d = nc.dram_tensor("glhs", [5, M], f16, kind="ExternalInput").ap()
    prhs_d = nc.dram_tensor("prhs", [5, NP], f16, kind="ExternalInput").ap()
    xx_d = nc.dram_tensor("xx", [PB, mb_blocks], f32, kind="ExternalInput").ap()
    ident_d = nc.dram_tensor("ident", [PB, PB], f16, kind="ExternalInput").ap()
    ones_d = nc.dram_tensor("ones", [PB, 1], f32, kind="ExternalInput").ap()
    out_d = nc.dram_tensor("out", [1, 1], f32, kind="ExternalOutput").ap()

    def body(ctx: ExitStack, tc: tile.TileContext):
        nc = tc.nc
        const_pool = ctx.enter_context(tc.tile_pool(name="const", bufs=1))
        acc_pool = ctx.enter_context(tc.tile_pool(name="acc", bufs=1))
        work_pool = ctx.enter_context(tc.tile_pool(name="work", bufs=2))
        psum_pool = ctx.enter_context(tc.tile_pool(name="psum", bufs=2, space="PSUM"))

        # K=5 operands replicated at partitions {0,32,64,96} so four
        # matmuls can run concurrently in distinct PE row groups
        glhs_sb = const_pool.tile([128, M], f16)
        prhs_sb = const_pool.tile([128, NP], f16)
        for r in range(4):
            nc.sync.dma_start(glhs_sb[32 * r : 32 * r + 5, :], glhs_d[:])
            nc.sync.dma_start(prhs_sb[32 * r : 32 * r + 5, :], prhs_d[:])
        xx_sb = const_pool.tile([PB, mb_blocks], f32)
        nc.sync.dma_start(xx_sb[:], xx_d[:])
        ident_sb = const_pool.tile([PB, PB], f16)
        nc.sync.dma_start(ident_sb[:], ident_d[:])
        ones_sb = const_pool.tile([PB, 1], f32)
        nc.sync.dma_start(ones_sb[:], ones_d[:])

        colacc = acc_pool.tile([PB, NP], f16)
        colaccB = (
            acc_pool.tile([PB, NP], f16, name="colaccB")
            if col_offload == "split"
            else None
        )
        rowmins = acc_pool.tile([PB, mb_blocks], f32)
        colmins = acc_pool.tile([PB, nch], f32)

        ident_act = mybir.ActivationFunctionType.Identity
        amin = mybir.AluOpType.min

        def mk_mm_noload(ps_slice, nb, r):
            # Hand-emitted non-self-loading InstMatmult: weights must have
            # been loaded into PE row group r by a prior ldweights() (same
            # engine, program order). CoreSim cannot simulate this form.
            rhs = prhs_sb[32 * r : 32 * r + 5, nb * FD : (nb + 1) * FD]
            te = nc.tensor
            ifmap_ap = te.lower_ap(rhs.opt({0}), opt=False)
            out_ap = te.lower_ap(ps_slice)
            return te.add_instruction(
                mybir.InstMatmult(
                    name=nc.get_next_instruction_name(),
                    replication_resolution=0,
                    replication_shift_amnt=0,
                    replication_num_rows=0,
                    start_tensor_calc=True,
                    stop_tensor_calc=True,
                    ins=[ifmap_ap],
                    outs=[out_ap],
                    perf_mode=None,
                    is_transpose=None,
                    ifmap_quant_offset=None,
                    weights_quant_offset=None,
                    bass_skip_group_check=True,
                    tile_position=(32 * r, 0),
                    tile_size=(32, 128),
                )
            )

        def mk_mm(ps_slice, mb, nb, j):
            r = j % 4 if rowgrp else 0
            nc.tensor.matmul(
                ps_slice,
                glhs_sb[32 * r : 32 * r + 5, mb * PB : (mb + 1) * PB],
                prhs_sb[32 * r : 32 * r + 5, nb * FD : (nb + 1) * FD],
                start=True,
                stop=True,
                tile_position=(32 * r, 0),
            )

        if classic:
            # v1 pipeline: per-512-tile ACT conversion + per-tile DVE mins
            for mb in range(mb_blocks):
                rowtile = work_pool.tile([PB, FD], f16, tag="rowtile", bufs=2)
                for nb in range(nb_blocks):
                    ps = psum_pool.tile([PB, FD], f32, tag="cps", bufs=4)
                    mk_mm(ps[:], mb, nb, nb)
                    if nb == 0:
                        dst = rowtile
                    else:
                        dst = work_pool.tile([PB, FD], f16, tag="stile", bufs=4)
                    nc.scalar.activation(
                        dst[:], ps[:], mybir.ActivationFunctionType.Identity,
                        bias=xx_sb[:, mb : mb + 1], scale=1.0,
                    )
                    csl = colacc[:, nb * FD : (nb + 1) * FD]
                    if mb == 0:
                        nc.vector.tensor_copy(csl, dst[:])
                    else:
                        nc.vector.tensor_tensor(csl, csl, dst[:], amin)
                    if nb > 0:
                        nc.vector.tensor_tensor(
                            rowtile[:], rowtile[:], dst[:], amin
                        )
                nc.vector.tensor_reduce(
                    rowmins[:, mb : mb + 1], rowtile[:],
                    axis=mybir.AxisListType.X, op=amin,
                )

        PSG = min(psg, nb_blocks)  # psum banks (512-col matmuls) per ACT op
        assert nb_blocks % PSG == 0

        if interleave and not classic:
            assert mb_blocks % 2 == 0
            for mbp in range(mb_blocks // 2):
                mbs = (2 * mbp, 2 * mbp + 1)
                rbufs = {}
                for mb in mbs:
                    rbufs[mb] = work_pool.tile(
                        [PB, NP], f16, tag="rowbuf", bufs=4, name=f"rowbuf{mb % 4}"
                    )
                for nbg in range(nb_blocks // PSG):
                    for mb in mbs:
                        ps = psum_pool.tile(
                            [PB, PSG * FD], f32, tag="ps", bufs=ps_bufs, name="ps"
                        )
                        for j in range(PSG):
                            nb = nbg * PSG + j
                            r = j % 4
                            nc.tensor.matmul(
                                ps[:, j * FD : (j + 1) * FD],
                                glhs_sb[32 * r : 32 * r + 5, mb * PB : (mb + 1) * PB],
                                prhs_sb[32 * r : 32 * r + 5, nb * FD : (nb + 1) * FD],
                                start=True,
                                stop=True,
                                tile_position=(32 * r, 0),
                            )
                        nc.scalar.activation(
                            rbufs[mb][:, nbg * PSG * FD : (nbg + 1) * PSG * FD],
                            ps[:],
                            ident_act,
                            bias=xx_sb[:, mb : mb + 1],
                            scale=1.0,
                        )
                for mb in mbs:
                    rowbuf = rbufs[mb]
                    if mb == 0:
                        nc.vector.tensor_copy(colacc[:], rowbuf[:])
                    else:
                        nc.vector.tensor_tensor(
                            colacc[:], colacc[:], rowbuf[:], amin
                        )
                    scr = work_pool.tile(
                        [PB, NP // 2], f16, tag="scr", bufs=work_bufs, name="scr"
                    )
                    nc.vector.tensor_tensor(
                        scr[:], rowbuf[:, : NP // 2], rowbuf[:, NP // 2 :], amin
                    )
                    w = NP // 4
                    while w > FD:
                        nc.vector.tensor_tensor(
                            scr[:, :w], scr[:, :w], scr[:, w : 2 * w], amin
                        )
                        w //= 2
                    nc.vector.tensor_tensor(
                        scr[:, :w], scr[:, :w], scr[:, w : 2 * w], amin
                    )
                    nc.vector.tensor_reduce(
                        rowmins[:, mb : mb + 1],
                        scr[:, :w],
                        axis=mybir.AxisListType.X,
                        op=amin,
                    )

        for mb in range(mb_blocks if not (classic or interleave) else 0):
            # one fp16 row-buffer holding this m-block's full distance row
            rowbuf = work_pool.tile([PB, NP], f16, tag="rowbuf", bufs=work_bufs)
            if preload:
                # load this m-block's weights into all four PE row groups
                # once; the matmuls below skip their weight reload
                for r in range(4):
                    nc.tensor.ldweights(
                        glhs_sb[32 * r : 32 * r + 5, mb * PB : (mb + 1) * PB],
                        tile_position=(32 * r, 0),
                    )
            for nbg in range(nb_blocks // PSG):
                ps = psum_pool.tile([PB, PSG * FD], f32, tag="ps", bufs=ps_bufs)
                for j in range(PSG * FD // mm_n):
                    off = nbg * PSG * FD + j * mm_n
                    r = j % 4 if rowgrp else 0
                    if preload:
                        mk_mm_noload(ps[:, j * FD : (j + 1) * FD], off // FD, r)
                        continue
                    nc.tensor.matmul(
                        ps[:, j * mm_n : (j + 1) * mm_n],
                        glhs_sb[32 * r : 32 * r + 5, mb * PB : (mb + 1) * PB],
                        prhs_sb[32 * r : 32 * r + 5, off : off + mm_n],
                        start=True,
                        stop=True,
                        tile_position=(32 * r, 0),
                    )
                # fp16 conversion + exact fp32 row bias |g|^2
                if finegrain and not skip_act:
                    GW = PSG * FD  # piece width (one ACT group)
                    lo, hi = nbg * GW, (nbg + 1) * GW
                    nc.scalar.activation(
                        rowbuf[:, lo:hi],
                        ps[:],
                        ident_act,
                        bias=xx_sb[:, mb : mb + 1],
                        scale=1.0,
                    )
                    # column piece as soon as this group lands
                    if not skip_col:
                        if mb == 0:
                            nc.vector.tensor_copy(
                                colacc[:, lo:hi], rowbuf[:, lo:hi]
                            )
                        else:
                            nc.vector.tensor_tensor(
                                colacc[:, lo:hi],
                                colacc[:, lo:hi],
                                rowbuf[:, lo:hi],
                                amin,
                            )
                    # tree level 1 pieces once both halves of a pair exist
                    if not skip_row:
                        half = NP // 2
                        if lo >= half or GW == NP:
                            if lo <= half:
                                scr_fg = work_pool.tile(
                                    [PB, NP // 2], f16, tag="scr",
                                    bufs=work_bufs, name="scrfg",
                                )
                            if GW == NP:
                                nc.vector.tensor_tensor(
                                    scr_fg[:], rowbuf[:, :half],
                                    rowbuf[:, half:], amin,
                                )
                            else:
                                plo = lo - half
                                nc.vector.tensor_tensor(
                                    scr_fg[:, plo : plo + GW],
                                    rowbuf[:, plo : plo + GW],
                                    rowbuf[:, lo:hi],
                                    amin,
                                )
                    continue
                if not skip_act:
                    if bias_imm:
                        nc.scalar.activation(
                            rowbuf[:, nbg * PSG * FD : (nbg + 1) * PSG * FD],
                            ps[:],
                            mybir.ActivationFunctionType.Copy,
                            bias=0.0,
                            scale=1.0,
                        )
                    else:
                        nc.scalar.activation(
                            rowbuf[:, nbg * PSG * FD : (nbg + 1) * PSG * FD],
                            ps[:],
                            ident_act,
                            bias=xx_sb[:, mb : mb + 1],
                            scale=1.0,
                        )
            if finegrain:
                # col pieces + tree level 1 already emitted per ACT group
                if skip_row:
                    continue
                scr = scr_fg
                w = NP // 4
                while w > FD:
                    nc.vector.tensor_tensor(
                        scr[:, :w], scr[:, :w], scr[:, w : 2 * w], amin
                    )
                    w //= 2
                nc.vector.tensor_tensor(
                    scr[:, :w], scr[:, :w], scr[:, w : 2 * w], amin
                )
                nc.vector.tensor_reduce(
                    rowmins[:, mb : mb + 1],
                    scr[:, :w],
                    axis=mybir.AxisListType.X,
                    op=amin,
                )
                continue
            # column path: one elementwise min over the whole row-buffer
            if skip_col:
                pass
            elif col_offload == "split" and mb % 2 == 1:
                # independent second chain on compute-DMA (SDMA CCE min)
                if mb == 1:
                    nc.gpsimd.dma_start(colaccB[:], rowbuf[:])
                else:
                    nc.gpsimd.dma_start(colaccB[:], rowbuf[:], accum_op=amin)
            elif mb == 0:
                nc.vector.tensor_copy(colacc[:], rowbuf[:])
            elif col_offload in ("gtt", "gdma") and mb % col_k == col_k - 1:
                if col_offload == "gtt":
                    nc.gpsimd.tensor_tensor(colacc[:], colacc[:], rowbuf[:], amin)
                else:
                    nc.gpsimd.dma_start(colacc[:], rowbuf[:], accum_op=amin)
            else:
                nc.vector.tensor_tensor(colacc[:], colacc[:], rowbuf[:], amin)
            # row path: in-place fp16 min-tree, final level fused with reduce
            if skip_row:
                continue
            scr = work_pool.tile([PB, NP // 2], f16, tag="scr", bufs=work_bufs)
            nc.vector.tensor_tensor(
                scr[:], rowbuf[:, : NP // 2], rowbuf[:, NP // 2 :], amin
            )
            w = NP // 4
            while w > FD:
                nc.vector.tensor_tensor(
                    scr[:, :w], scr[:, :w], scr[:, w : 2 * w], amin
                )
                w //= 2
            if use_ttr:
                nc.vector.tensor_tensor_reduce(
                    out=scr[:, :w],
                    in0=scr[:, :w],
                    in1=scr[:, w : 2 * w],
                    scale=1.0,
                    scalar=60000.0,
                    op0=amin,
                    op1=amin,
                    accum_out=rowmins[:, mb : mb + 1],
                )
            else:
                nc.vector.tensor_tensor(
                    scr[:, :w], scr[:, :w], scr[:, w : 2 * w], amin
                )
                nc.vector.tensor_reduce(
                    rowmins[:, mb : mb + 1],
                    scr[:, :w],
                    axis=mybir.AxisListType.X,
                    op=amin,
                )

        # merge the second column chain (if split)
        if colaccB is not None and not skip_col:
            nc.vector.tensor_tensor(colacc[:], colacc[:], colaccB[:], amin)

        # partition-axis min of colacc via PE transposes
        if not skip_col and not skip_act:
            for ch in range(nch):
                tps = psum_pool.tile([PB, PB], f16, tag="ps", bufs=ps_bufs)
                nc.tensor.transpose(
                    tps[:], colacc[:, ch * PB : (ch + 1) * PB], ident_sb[:]
                )
                nc.vector.tensor_reduce(
                    colmins[:, ch : ch + 1],
                    tps[:],
                    axis=mybir.AxisListType.X,
                    op=mybir.AluOpType.min,
                )

        rsum = acc_pool.tile([PB, 1], f32)
        if not skip_row and not skip_act:
            nc.vector.tensor_reduce(
                rsum[:],
                rowmins[:],
                axis=mybir.AxisListType.X,
                op=mybir.AluOpType.add,
            )
        else:
            nc.vector.tensor_copy(rsum[:], ones_sb[:])
        csum = acc_pool.tile([PB, 1], f32)
        if not skip_col and not skip_act:
            nc.vector.tensor_reduce(
                csum[:],
                colmins[:],
                axis=mybir.AxisListType.X,
                op=mybir.AluOpType.add,
            )
        else:
            nc.vector.tensor_copy(csum[:], ones_sb[:])
        tot = acc_pool.tile([PB, 1], f32)
        nc.vector.tensor_add(tot[:], rsum[:], csum[:])
        outp = psum_pool.tile([1, 1], f32, tag="ps", bufs=ps_bufs)
        nc.tensor.matmul(outp[:], tot[:], ones_sb[:], start=True, stop=True)
        out_sb = acc_pool.tile([1, 1], f32)
        nc.vector.tensor_copy(out_sb[:], outp[:])
        nc.sync.dma_start(out_d[:], out_sb[:])

    with tile.TileContext(nc) as tc:
        with ExitStack() as ctx:
            if loop > 1:
                with tc.For_i(0, loop, 1):
                    body(ctx, tc)
            else:
                body(ctx, tc)

    nc.compile()
    return nc


def _get_nc(mb_blocks=MB, nb_blocks=NB):
    key = (mb_blocks, nb_blocks)
    if key not in _CACHE:
        _CACHE[key] = _build(mb_blocks, nb_blocks)
    return _CACHE[key]


def _prep_core(g, p, mb_blocks, nb_blocks):
    """Host-side prep of one core's input arrays from gts[b], preds[b]."""
    m = mb_blocks * PB
    n = nb_blocks * FD
    g16 = g[:m].astype(np.float16)
    p16 = p[:n].astype(np.float16)
    g32 = g16.astype(np.float32)
    p32 = p16.astype(np.float32)

    glhs = np.empty((5, m), np.float16)
    glhs[0:3] = (-2.0 * g32.T).astype(np.float16)  # exact: *2 bumps exponent
    glhs[3:5] = np.float16(1.0)

    yy = (p32 * p32).sum(-1, dtype=np.float32)
    yy_hi = yy.astype(np.float16)
    yy_lo = (yy - yy_hi.astype(np.float32)).astype(np.float16)
    prhs = np.empty((5, n), np.float16)
    prhs[0:3] = p16.T
    prhs[3] = yy_hi
    prhs[4] = yy_lo

    xx = (g32 * g32).sum(-1, dtype=np.float32)
    xx_in = np.ascontiguousarray(xx.reshape(mb_blocks, PB).T)  # [128, MB]

    return {
        "glhs": glhs,
        "prhs": prhs,
        "xx": xx_in,
        "ident": np.eye(PB, dtype=np.float16),
        "ones": np.ones((PB, 1), np.float32),
    }


def kernel(preds, gts):
    preds = np.asarray(preds)
    gts = np.asarray(gts)
    assert preds.shape == (B, NPTS, D) and gts.shape == (B, NPTS, D)

    from concourse.bass_utils import run_bass_kernel_spmd

    nc = _get_nc()
    in_maps = [_prep_core(gts[b], preds[b], MB, NB) for b in range(B)]
    res = run_bass_kernel_spmd(nc, in_maps, list(range(B)))
    total = sum(float(r["out"][0, 0]) for r in res.results)
    # loss = sum(colmins)/(B*N) + sum(rowmins)/(B*M), N == M == NPTS
    return np.float32(total / (B * NPTS))
